# revision 28
# baseline (speedup 1.0000x reference)
"""GNN message-passing (3x GraphSAGE-mean + GraphNorm + ReLU, then 3 MLP heads)
on 8 trn2 NeuronCores.

Sharding: node partition. Core c owns nodes [c*NPC, (c+1)*NPC); weights are
replicated. Per layer each core:
  1. gathers h[src] rows for its in-edges out of a full DRAM replica of h with
     dma_gather (int16 indices => the source is addressed in chunks of 30000
     rows), edges pre-sorted by (dst-tile, src-chunk) and padded to 128-slot
     groups on the host;
  2. aggregates each 128-node dst tile with a one-hot matmul whose values are
     1/deg (built on-chip from iota==dst_local times 1/deg), accumulating
     agg^T (feature-major) in PSUM across the source chunks;
  3. z^T = Wl @ agg^T + Wr @ h_own^T + bl, all feature-major, so GraphNorm
     batch stats are free-axis reductions and scale/bias/ReLU fuse into one
     scalar-engine activation;
  4. GraphNorm stats: per-core partial sums -> 1KB AllReduce -> finalize;
  5. post-norm shard is PE-transposed to node-major and AllGather'd into the
     next layer's replica (skipped after layer 2, where the MLP heads consume
     the feature-major tiles directly; head outputs are packed 8+5+9=22 wide).
"""
import sys

sys.path.insert(0, "/opt/trn_rl_repo")

import numpy as np

import concourse.bass as bass
import concourse.bacc as bacc
import concourse.mybir as mybir
import concourse.tile as tile

FP = mybir.dt.float32
I16 = mybir.dt.int16
AL = mybir.AluOpType
AF = mybir.ActivationFunctionType


class Cfg:
    def __init__(self, N=300000, NH=250000, IN_C=32, HID=128, CORES=8,
                 CHUNK=30000, W=10, MLP_H=256, EPS=1e-5, STAGE=99):
        self.STAGE = STAGE   # debug bisection: 1=agg only, 2=+z, 3=+AR, 4=+norm, 5=+AG, 99=full
        self.SUB = "full"    # debug: 'g'=gathers only, 'm'=+M build, 'full'
        assert N % CORES == 0
        self.N, self.NH, self.IN_C, self.HID = N, NH, IN_C, HID
        self.CORES, self.CHUNK, self.W = CORES, CHUNK, W
        self.MLP_H, self.EPS = MLP_H, EPS
        self.OUT = 22                               # 8 + 5 + 9 packed heads
        self.NPC = N // CORES
        self.NT = (self.NPC + 127) // 128           # node tiles per core
        self.NPCP = self.NT * 128                   # padded own nodes
        self.NK = (N + CHUNK - 1) // CHUNK          # src chunks (int16 range)
        assert CHUNK <= 32768
        self.XPADC = 64                             # x row padded to 64 f32 (256B)
        self.NMLPC = 3 * MLP_H // 128               # MLP hidden chunks (6)
        self.wins = []
        j = 0
        while j < self.NT:
            self.wins.append((j, min(j + W, self.NT)))
            j += W


def preprocess(cfg, src, dst):
    """Host index preprocessing: per-core gather indices + one-hot metadata,
    plus the (core-independent) block structure baked into the program."""
    C, NPC, NT, NK, CHUNK = cfg.CORES, cfg.NPC, cfg.NT, cfg.NK, cfg.CHUNK
    src = np.asarray(src, np.int64)
    dst = np.asarray(dst, np.int64)

    deg = np.bincount(dst, minlength=cfg.N)
    recip_deg = (1.0 / np.maximum(deg, 1.0)).astype(np.float32)

    owner = dst // NPC
    per_core = []
    counts = np.zeros((C, NT, NK), np.int64)
    for c in range(C):
        m = owner == c
        s, d = src[m], dst[m]
        dl = d - c * NPC
        j = dl >> 7
        k = s // CHUNK
        key = j * NK + k
        order = np.lexsort((s, key))
        s, d, key = s[order], d[order], key[order]
        counts[c] = np.bincount(key, minlength=NT * NK).reshape(NT, NK)
        per_core.append(dict(s=s, d=d, key=key))

    maxcnt = counts.max(axis=0)
    blocks = ((maxcnt + 127) // 128).astype(np.int64)     # [NT, NK]

    bj = blocks.sum(axis=1)
    jb_base = np.zeros(NT + 1, np.int64)
    jb_base[1:] = np.cumsum(bj)
    NB = int(jb_base[-1])
    NSLOT = NB * 128

    # gather-slot layout: for window w: for k: for j in w: blocks[j,k] blocks
    slot_base = np.zeros((NT, NK), np.int64)
    gsec = []                                             # [win][k] = (slot_off, nblocks)
    off = 0
    for (j0, j1) in cfg.wins:
        secs = []
        for k in range(NK):
            nb = int(blocks[j0:j1, k].sum())
            run = 0
            for j in range(j0, j1):
                slot_base[j, k] = off + run * 128
                run += int(blocks[j, k])
            secs.append((off, nb))
            off += nb * 128
        gsec.append(secs)
    assert off == NSLOT

    # j-major block index of each (j,k) group (for dstl/recip/M ordering)
    jm_base = np.zeros((NT, NK), np.int64)
    for j in range(NT):
        jm_base[j] = jb_base[j] + np.concatenate(([0], np.cumsum(blocks[j])[:-1]))

    cores = []
    for c in range(C):
        pc = per_core[c]
        key = pc["key"]
        grp_start = np.zeros(NT * NK, np.int64)
        grp_start[1:] = np.cumsum(np.bincount(key, minlength=NT * NK))[:-1]
        rank = np.arange(len(key)) - grp_start[key]
        jj = key // NK
        kk = key % NK
        gslot = slot_base[jj, kk] + rank
        mslot = jm_base[jj, kk] * 128 + rank

        gidx = np.zeros(NSLOT, np.int16)
        gidx[gslot] = (pc["s"] - kk * CHUNK).astype(np.int16)
        dstl = np.zeros(NSLOT, np.float32)
        dstl[mslot] = ((pc["d"] - c * NPC) & 127).astype(np.float32)
        recip = np.zeros(NSLOT, np.float32)
        recip[mslot] = recip_deg[pc["d"]]

        cores.append(dict(
            # [128, NSLOT/16]: 16-partition wrap replicated 8x down partitions
            gidx=np.ascontiguousarray(np.tile(gidx.reshape(-1, 16).T, (8, 1))),
            dstl=np.ascontiguousarray(dstl.reshape(NB, 128).T),   # [128, NB]
            recip=np.ascontiguousarray(recip.reshape(NB, 128).T),
        ))

    meta = dict(blocks=blocks, jb_base=jb_base, jm_base=jm_base,
                gsec=gsec, NB=NB, NSLOT=NSLOT,
                NBW=max(int(jb_base[j1] - jb_base[j0]) for j0, j1 in cfg.wins),
                BJMAX=int(bj.max()))
    return meta, cores


def build(cfg, meta):
    """Build the SPMD Bass program (identical for all cores)."""
    N, NT, NK, HID, NPC = cfg.N, cfg.NT, cfg.NK, cfg.HID, cfg.NPC
    NB, NBW, BJMAX = meta["NB"], meta["NBW"], meta["BJMAX"]
    blocks, jb_base, jm_base, gsec = (meta["blocks"], meta["jb_base"],
                                      meta["jm_base"], meta["gsec"])
    wins = cfg.wins
    LASTV = 128 - (cfg.NPCP - NPC)                  # valid nodes in last tile
    nw = len(wins)

    nc = bacc.Bacc(trn_type="TRN2", target_bir_lowering=False,
                   num_devices=cfg.CORES)
    RG = [list(range(cfg.CORES))]

    def dip(name, shape, dt=FP):
        return nc.declare_dram_parameter(name, list(shape), dt, isOutput=False)

    xpad = dip("xpad", (N, cfg.XPADC))
    xT = dip("xT", (cfg.IN_C, cfg.NPCP))
    gidx_d = dip("gidx", (128, meta["NSLOT"] // 16), I16)
    dstl_d = dip("dstl", (128, NB))
    recip_d = dip("recip", (128, NB))
    iota_d = dip("iota", (128, 128))
    ident_d = dip("ident", (128, 128))
    wT = {}
    for li in range(3):
        kin = cfg.IN_C if li == 0 else HID
        wT[f"wl{li}"] = dip(f"wl{li}T", (kin, HID))
        wT[f"wr{li}"] = dip(f"wr{li}T", (kin, HID))
    lprm = [dip(f"lprm{li}", (128, 5)) for li in range(3)]   # bl,alpha,a2,wn,bn
    w1T_d = dip("w1T", (HID, 3 * cfg.MLP_H))
    b1_d = dip("b1", (128, cfg.NMLPC))
    w2T_d = dip("w2T", (128, cfg.NMLPC * cfg.OUT))
    b2_d = dip("b2", (128, 1))

    heads_out = nc.declare_dram_parameter("heads", [cfg.NPCP, cfg.OUT], FP,
                                          isOutput=True)

    shr = "Shared" if cfg.CORES > 4 else "Local"
    zT_own = nc.dram_tensor("zT_own", [128, cfg.NPCP], FP)
    hT_own = [nc.dram_tensor(f"hT_own{l}", [128, cfg.NPCP], FP) for l in range(2)]
    agin = [nc.dram_tensor(f"agin{l}", [NPC, HID], FP) for l in range(2)]
    agout = [nc.dram_tensor(f"agout{l}", [N, HID], FP, addr_space=shr)
             for l in range(2)]
    arin = [nc.dram_tensor(f"arin{l}", [128, 2], FP) for l in range(3)]
    arout = [nc.dram_tensor(f"arout{l}", [128, 2], FP, addr_space=shr)
             for l in range(3)]

    with tile.TileContext(nc) as tc:
        with (
            tc.tile_pool(name="const", bufs=1) as cpool,
            tc.tile_pool(name="gbuf", bufs=2) as gpool,
            tc.tile_pool(name="meta", bufs=2) as mpool,
            tc.tile_pool(name="mtile", bufs=2) as mtpool,
            tc.tile_pool(name="stage", bufs=2) as spool,
            tc.tile_pool(name="small", bufs=2) as smpool,
            tc.tile_pool(name="ps", bufs=3, space="PSUM") as psP,
            tc.tile_pool(name="ps2", bufs=2, space="PSUM") as psQ,
        ):
            def load_const(name, dram, shape):
                t = cpool.tile(list(shape), FP, tag=name)
                nc.sync.dma_start(t[:], dram[:])
                return t

            iota_s = load_const("iota", iota_d, (128, 128))
            ident_s = load_const("ident", ident_d, (128, 128))
            wT_s = {}
            for li in range(3):
                kin = cfg.IN_C if li == 0 else HID
                for nm in (f"wl{li}", f"wr{li}"):
                    wT_s[nm] = load_const(nm, wT[nm], (kin, HID))
            lprm_s = [load_const(f"lprm{li}", lprm[li], (128, 5)) for li in range(3)]
            w1T_s = load_const("w1T", w1T_d, (HID, 3 * cfg.MLP_H))
            b1_s = load_const("b1", b1_d, (128, cfg.NMLPC))
            w2T_s = load_const("w2T", w2T_d, (128, cfg.NMLPC * cfg.OUT))
            b2_s = load_const("b2", b2_d, (128, 1))
            stat_s = cpool.tile([128, 2 * nw], FP, tag="stat")
            stat_f = cpool.tile([128, 8], FP, tag="statf")
            eps_s = cpool.tile([128, 1], FP, tag="eps")
            nc.vector.memset(eps_s[:], float(cfg.EPS))

            _regs = {}

            def getreg(v):
                if v not in _regs:
                    _regs[v] = nc.gpsimd.to_reg(v)
                return _regs[v]

            def bc3(ap2, nb):
                """[128, nb] -> broadcast [128, nb, 128] along a new last axis."""
                return ap2.rearrange("p (b o) -> p b o", o=1).to_broadcast([128, nb, 128])

            def agg_window(wi, elem, feat_m, src_dram):
                """Gather + aggregate window wi. Returns aggT [feat_m, W*128]."""
                j0, j1 = wins[wi]
                jb0 = int(jb_base[j0])
                nbw = int(jb_base[j1]) - jb0
                w0 = gsec[wi][0][0]                  # window slot offset
                g = gpool.tile([128, NBW, elem], FP, tag="G")
                idxw = mpool.tile([128, NBW * 8], I16, tag="idx")
                dstw = mpool.tile([128, NBW], FP, tag="dst")
                recw = mpool.tile([128, NBW], FP, tag="rec")
                nc.sync.dma_start(idxw[:, : nbw * 8],
                                  gidx_d[:, w0 // 16: (w0 + nbw * 128) // 16])
                nc.sync.dma_start(dstw[:, :nbw], dstl_d[:, jb0: jb0 + nbw])
                nc.sync.dma_start(recw[:, :nbw], recip_d[:, jb0: jb0 + nbw])
                for k in range(NK):
                    off, nb = gsec[wi][k]
                    if nb == 0:
                        continue
                    lob = (off - w0) // 128
                    nc.gpsimd.dma_gather(
                        out_ap=g[:, lob: lob + nb, :],
                        in_ap=src_dram[k * cfg.CHUNK: min((k + 1) * cfg.CHUNK, N), :],
                        idxs_ap=idxw[:, lob * 8: (lob + nb) * 8],
                        num_idxs=nb * 128,
                        num_idxs_reg=getreg(nb * 128),
                        elem_size=elem,
                        single_packet=False,
                    )
                aggT = spool.tile([128, cfg.W * 128], FP, tag="aggT")
                if cfg.SUB == "g":
                    nc.vector.memset(aggT[:], 0.0)
                    return aggT
                for j in range(j0, j1):
                    c0 = (j - j0) * 128
                    bjn = int(blocks[j].sum())
                    if bjn == 0:
                        nc.vector.memset(aggT[:feat_m, c0: c0 + 128], 0.0)
                        continue
                    mb0 = int(jm_base[j, 0]) - jb0
                    m = mtpool.tile([128, BJMAX, 128], FP, tag="M")
                    nc.vector.tensor_tensor(
                        out=m[:, :bjn, :],
                        in0=iota_s[:].rearrange("p (o f) -> p o f", o=1)
                                     .to_broadcast([128, bjn, 128]),
                        in1=bc3(dstw[:, mb0: mb0 + bjn], bjn),
                        op=AL.is_equal,
                    )
                    nc.vector.tensor_tensor(
                        out=m[:, :bjn, :], in0=m[:, :bjn, :],
                        in1=bc3(recw[:, mb0: mb0 + bjn], bjn),
                        op=AL.mult,
                    )
                    if cfg.SUB == "m":
                        nc.vector.memset(aggT[:feat_m, c0: c0 + 128], 0.0)
                        continue
                    ps = psP.tile([128, 128], FP, tag="ps128")
                    mi = 0
                    for k in range(NK):
                        nbk = int(blocks[j, k])
                        if nbk == 0:
                            continue
                        goff = (gsec[wi][k][0] - w0) // 128 + int(blocks[j0:j, k].sum())
                        for b in range(nbk):
                            nc.tensor.matmul(
                                out=ps[:feat_m, :],
                                lhsT=g[:, goff + b, :feat_m],
                                rhs=m[:, mi, :],
                                start=(mi == 0),
                                stop=(mi == bjn - 1),
                            )
                            mi += 1
                    nc.vector.tensor_copy(aggT[:feat_m, c0: c0 + 128],
                                          ps[:feat_m, :])
                return aggT

            def heads_window(wi, j0, j1, hst):
                wn = j1 - j0
                outst = spool.tile([128, cfg.W, cfg.OUT], FP, tag="outst")
                for j in range(j0, j1):
                    c0 = (j - j0) * 128
                    ps2 = psQ.tile([cfg.OUT, 128], FP, tag="mlp2")
                    for h in range(cfg.NMLPC):
                        ps = psP.tile([128, 128], FP, tag="ps128")
                        nc.tensor.matmul(out=ps[:],
                                         lhsT=w1T_s[:, h * 128:(h + 1) * 128],
                                         rhs=hst[:, c0: c0 + 128],
                                         start=True, stop=True)
                        a1 = smpool.tile([128, 128], FP, tag="a1")
                        nc.scalar.activation(a1[:], ps[:], AF.Relu,
                                             bias=b1_s[:, h: h + 1], scale=1.0)
                        nc.tensor.matmul(out=ps2[:],
                                         lhsT=w2T_s[:, h * cfg.OUT:(h + 1) * cfg.OUT],
                                         rhs=a1[:],
                                         start=(h == 0), stop=(h == cfg.NMLPC - 1))
                    ot = smpool.tile([cfg.OUT, 128], FP, tag="outT")
                    nc.vector.tensor_scalar(out=ot[:], in0=ps2[:],
                                            scalar1=b2_s[: cfg.OUT, 0:1],
                                            scalar2=None, op0=AL.add)
                    ps3 = psQ.tile([128, cfg.OUT], FP, tag="tr2")
                    nc.tensor.transpose(out=ps3[:], in_=ot[:],
                                        identity=ident_s[: cfg.OUT, : cfg.OUT])
                    nc.vector.tensor_copy(outst[:, j - j0, :], ps3[:])
                nc.sync.dma_start(
                    heads_out[j0 * 128: j1 * 128, :].rearrange(
                        "(b p) f -> p b f", p=128),
                    outst[:, :wn, :],
                )

            def layer(li):
                elem = cfg.XPADC if li == 0 else HID
                feat_m = cfg.XPADC if li == 0 else HID
                kin = cfg.IN_C if li == 0 else HID
                src_dram = xpad if li == 0 else agout[li - 1]
                root_dram = xT if li == 0 else hT_own[li - 1]
                pl = lprm_s[li]
                # ---- phase 1: z^T windows -> DRAM, stats partials ----
                for wi, (j0, j1) in enumerate(wins):
                    wn = j1 - j0
                    aggT = agg_window(wi, elem, feat_m, src_dram)
                    rootw = spool.tile([kin, cfg.W * 128], FP, tag="root")
                    nc.sync.dma_start(rootw[:, : wn * 128],
                                      root_dram[:kin, j0 * 128: j1 * 128])
                    zst = spool.tile([128, cfg.W * 128], FP, tag="zst")
                    for j in range(j0, j1):
                        c0 = (j - j0) * 128
                        ps = psP.tile([128, 128], FP, tag="ps128")
                        nc.tensor.matmul(out=ps[:], lhsT=wT_s[f"wl{li}"][:],
                                         rhs=aggT[:kin, c0: c0 + 128],
                                         start=True, stop=False)
                        nc.tensor.matmul(out=ps[:], lhsT=wT_s[f"wr{li}"][:],
                                         rhs=rootw[:, c0: c0 + 128],
                                         start=False, stop=True)
                        nc.vector.tensor_scalar(out=zst[:, c0: c0 + 128],
                                                in0=ps[:], scalar1=pl[:, 0:1],
                                                scalar2=None, op0=AL.add)
                    vcols = wn * 128 if j1 < NT else (wn - 1) * 128 + LASTV
                    nc.vector.tensor_reduce(out=stat_s[:, 2 * wi: 2 * wi + 1],
                                            in_=zst[:, :vcols],
                                            axis=mybir.AxisListType.X, op=AL.add)
                    sq = spool.tile([128, cfg.W * 128], FP, tag="scr")
                    nc.scalar.square(sq[:, :vcols], zst[:, :vcols])
                    nc.vector.tensor_reduce(out=stat_s[:, 2 * wi + 1: 2 * wi + 2],
                                            in_=sq[:, :vcols],
                                            axis=mybir.AxisListType.X, op=AL.add)
                    nc.sync.dma_start(zT_own[:, j0 * 128: j1 * 128],
                                      zst[:, : wn * 128])
                if cfg.STAGE <= 1:
                    return
                # ---- stats AllReduce + finalize s,t ----
                st3 = stat_s[:].rearrange("p (w t) -> p w t", t=2)
                nc.vector.tensor_reduce(out=stat_f[:, 0:1], in_=st3[:, :, 0:1],
                                        axis=mybir.AxisListType.XY, op=AL.add)
                nc.vector.tensor_reduce(out=stat_f[:, 1:2], in_=st3[:, :, 1:2],
                                        axis=mybir.AxisListType.XY, op=AL.add)
                nc.sync.dma_start(arin[li][:], stat_f[:, 0:2])
                nc.gpsimd.collective_compute(
                    "AllReduce", AL.add, replica_groups=RG,
                    ins=[arin[li].ap().opt()], outs=[arout[li].ap().opt()],
                )
                nc.sync.dma_start(stat_f[:, 0:2], arout[li][:])
                inv_n = 1.0 / float(N)
                mu, m2 = stat_f[:, 2:3], stat_f[:, 3:4]
                var, rstd = stat_f[:, 4:5], stat_f[:, 5:6]
                sv, tv = stat_f[:, 6:7], stat_f[:, 7:8]
                nc.vector.tensor_scalar(out=mu, in0=stat_f[:, 0:1],
                                        scalar1=inv_n, scalar2=None, op0=AL.mult)
                nc.vector.tensor_scalar(out=m2, in0=stat_f[:, 1:2],
                                        scalar1=inv_n, scalar2=None, op0=AL.mult)
                nc.vector.tensor_tensor(out=var, in0=mu, in1=mu, op=AL.mult)
                nc.vector.tensor_tensor(out=var, in0=var, in1=pl[:, 2:3], op=AL.mult)
                nc.vector.tensor_tensor(out=var, in0=m2, in1=var, op=AL.subtract)
                nc.scalar.activation(rstd, var, AF.Sqrt, bias=eps_s[:, 0:1],
                                     scale=1.0)
                nc.vector.reciprocal(rstd, rstd)
                nc.vector.tensor_tensor(out=sv, in0=pl[:, 3:4], in1=rstd, op=AL.mult)
                nc.vector.tensor_tensor(out=tv, in0=pl[:, 1:2], in1=mu, op=AL.mult)
                nc.vector.tensor_tensor(out=tv, in0=sv, in1=tv, op=AL.mult)
                nc.vector.tensor_tensor(out=tv, in0=pl[:, 4:5], in1=tv, op=AL.subtract)
                if cfg.STAGE <= 2:
                    return
                # ---- phase 2: normalize + ReLU, transpose, AllGather / heads ----
                for wi, (j0, j1) in enumerate(wins):
                    wn = j1 - j0
                    zst = spool.tile([128, cfg.W * 128], FP, tag="zst")
                    nc.sync.dma_start(zst[:, : wn * 128],
                                      zT_own[:, j0 * 128: j1 * 128])
                    nc.scalar.activation(zst[:, : wn * 128], zst[:, : wn * 128],
                                         AF.Relu, bias=tv, scale=sv)
                    if li < 2:
                        nc.sync.dma_start(hT_own[li][:, j0 * 128: j1 * 128],
                                          zst[:, : wn * 128])
                        nm = spool.tile([128, cfg.W, HID], FP, tag="scr")
                        for j in range(j0, j1):
                            pst = psP.tile([128, 128], FP, tag="ps128")
                            nc.tensor.transpose(
                                out=pst[:],
                                in_=zst[:, (j - j0) * 128: (j - j0 + 1) * 128],
                                identity=ident_s[:])
                            nc.vector.tensor_copy(nm[:, j - j0, :], pst[:])
                        nfull = wn if j1 * 128 <= NPC else wn - 1
                        if nfull > 0:
                            nc.sync.dma_start(
                                agin[li][j0 * 128: j0 * 128 + nfull * 128, :]
                                .rearrange("(b p) f -> p b f", p=128),
                                nm[:, :nfull, :])
                        if nfull < wn:
                            nc.sync.dma_start(
                                agin[li][(j1 - 1) * 128: NPC, :],
                                nm[:LASTV, wn - 1, :])
                    else:
                        heads_window(wi, j0, j1, zst)
                if li < 2 and cfg.STAGE >= 4:
                    nc.gpsimd.collective_compute(
                        "AllGather", AL.bypass, replica_groups=RG,
                        ins=[agin[li].ap().opt()], outs=[agout[li].ap().opt()],
                    )

            layer(0)
            if cfg.STAGE >= 5:
                layer(1)
            if cfg.STAGE >= 6:
                layer(2)

    return nc


def make_inputs(cfg, meta, cores, x, params):
    N, HID = cfg.N, cfg.HID
    x = np.asarray(x, np.float32)
    xpad = np.zeros((N, cfg.XPADC), np.float32)
    xpad[:, : cfg.IN_C] = x
    iota = np.ascontiguousarray(
        np.broadcast_to(np.arange(128, dtype=np.float32), (128, 128)))
    ident = np.eye(128, dtype=np.float32)

    def W(a):
        return np.ascontiguousarray(np.asarray(a, np.float32))

    common = dict(xpad=xpad, iota=iota, ident=ident)
    for li in range(3):
        p = params[f"conv{li}"]
        common[f"wl{li}T"] = np.ascontiguousarray(W(p["Wl"]).T)
        common[f"wr{li}T"] = np.ascontiguousarray(W(p["Wr"]).T)
        n = params[f"norm{li}"]
        alpha = W(n["alpha"]).reshape(-1)
        lp = np.zeros((128, 5), np.float32)
        lp[:HID, 0] = W(p["bl"]).reshape(-1)
        lp[:HID, 1] = alpha
        lp[:HID, 2] = 2 * alpha - alpha * alpha
        lp[:HID, 3] = W(n["w"]).reshape(-1)
        lp[:HID, 4] = W(n["b"]).reshape(-1)
        common[f"lprm{li}"] = lp
    # heads packed (hh, eth, rel): hidden = concat of the three 256-wide MLPs
    heads = [params["mlp_hh"], params["mlp_eth"], params["mlp_rel"]]
    w1 = np.concatenate([W(h["W1"]) for h in heads], axis=0)       # [768, HID]
    common["w1T"] = np.ascontiguousarray(w1.T)                      # [HID, 768]
    b1 = np.concatenate([W(h["b1"]).reshape(-1) for h in heads])    # [768]
    common["b1"] = np.ascontiguousarray(b1.reshape(cfg.NMLPC, 128).T)
    # block-diagonal W2: [22, 768]
    w2 = np.zeros((cfg.OUT, 3 * cfg.MLP_H), np.float32)
    r = 0
    for hi, h in enumerate(heads):
        w2h = W(h["W2"])
        w2[r: r + w2h.shape[0], hi * cfg.MLP_H:(hi + 1) * cfg.MLP_H] = w2h
        r += w2h.shape[0]
    w2T = np.zeros((128, cfg.NMLPC * cfg.OUT), np.float32)
    for h in range(cfg.NMLPC):
        w2T[:, h * cfg.OUT:(h + 1) * cfg.OUT] = w2[:, h * 128:(h + 1) * 128].T
    common["w2T"] = w2T
    b2 = np.concatenate([W(h["b2"]).reshape(-1) for h in heads])
    b2p = np.zeros((128, 1), np.float32)
    b2p[: cfg.OUT, 0] = b2
    common["b2"] = b2p

    xT_full = np.ascontiguousarray(x.T)                             # [32, N]
    in_maps = []
    for c in range(cfg.CORES):
        xt = np.zeros((cfg.IN_C, cfg.NPCP), np.float32)
        xt[:, : cfg.NPC] = xT_full[:, c * cfg.NPC:(c + 1) * cfg.NPC]
        m = dict(common)
        m["xT"] = xt
        m["gidx"] = cores[c]["gidx"]
        m["dstl"] = cores[c]["dstl"]
        m["recip"] = cores[c]["recip"]
        in_maps.append(m)
    return in_maps


def assemble(cfg, results):
    """results: list per core of dict with 'heads' [NPCP, 22]."""
    full = np.concatenate([np.asarray(r["heads"])[: cfg.NPC] for r in results],
                          axis=0)[: cfg.NH]
    return (np.ascontiguousarray(full[:, 0:8]),
            np.ascontiguousarray(full[:, 8:13]),
            np.ascontiguousarray(full[:, 13:22]))


_BUILD_CACHE = {}


def run(x, src, dst, params, cfg=None, trace=False):
    from concourse.bass_utils import run_bass_kernel_spmd
    cfg = cfg or Cfg()
    meta, cores = preprocess(cfg, src, dst)
    nc = build(cfg, meta)
    nc.finalize()
    in_maps = make_inputs(cfg, meta, cores, x, params)
    res = run_bass_kernel_spmd(nc, in_maps, core_ids=list(range(cfg.CORES)),
                               trace=trace)
    out = assemble(cfg, res.results)
    return out, res


def kernel(x, src, dst, params):
    out, _ = run(x, src, dst, params)
    return out
